# revision 3
# baseline (speedup 1.0000x reference)
"""AttnBlock via partially-PE-folded Winograd F(2x2,3x3) qkv conv on 8 TRN2
NeuronCores, data parallel (2 samples per core).

Winograd decomposition: y = A^T [ (G w G^T) . (B^T d B) ] A. Host precomputes
the weight transform and input transform. On device, the px-contraction of the
output transform (A-columns) is folded INTO the PE via sign-folded weight
copies: for each output column chain ox
    chain c0 (ox=0): +W(py,0) +W(py,1) +W(py,2)
    chain c1 (ox=1): +W(py,1) -W(py,2) -W(py,3)
accumulate in PSUM (5 unique weight slots: W0,W1,W2,-W2,-W3). This costs 1.5x
the pure-Winograd GEMM MACs (still 1.5x fewer than direct conv) but removes
two thirds of the vector-engine transform work, which measured ~0.8-1.5us per
[128,512] op and otherwise starves the PE. The remaining py-contraction
(A-rows [1,1,1,0] / [0,1,-1,-1]) runs per unit on ScalarE (2 PSUM evictions)
+ DVE (3 ops) + GpSimd (1 op).

Other structure (validated in CoreSim against a numpy golden model):
  - v GEMMs run transposed (lhsT = X-tilde, rhs = W-tilde_v) so vT lands
    [pix, co] directly; no PE transposes.
  - Winograd pixel order (s; oyox, ty, tx) kept through attention (softmax and
    1x1 proj are permutation invariant), undone on the host gather.
  - qkv bias folds: k-bias cancels in softmax; v-bias -> host-side constant
    w_proj @ b_v; q-bias -> per-m-chunk pre-exp bias via K=512,N=1 matmuls.
  - scores/h_un/proj accumulate into 2-bank [P,1024] PSUM tiles so each
    ScalarE ACTIVATE drains both N-tiles in one op.
  - All input DMAs ride the sync-engine HWDGE queue (gpsimd must stay free
    for transform ops; SWDGE issue was observed to serialize at ~2.4us/MB).
"""

import numpy as np
import ml_dtypes

import concourse.bass as bass
import concourse.tile as tile
from concourse import bacc, mybir
from concourse.bass_utils import run_bass_kernel_spmd

P = 128
B, C, H, W = 16, 512, 32, 32
NCORES = 8
S = B // NCORES      # samples per core
T = 256              # winograd tiles per sample (16x16)
N = S * T            # 512 gemm columns per point
NPIX = 1024

BF16 = mybir.dt.bfloat16
F32 = mybir.dt.float32
F8 = mybir.dt.float8e4
EXP = mybir.ActivationFunctionType.Exp
DR = mybir.MatmulPerfMode.DoubleRow
SEXP = (C ** -0.5) / 1024.0   # exp scale: q8,k8 both carry x32

TRACE = False
LAST_EXEC_NS = None
_CACHED = None

Bt_np = np.array([[1, 0, -1, 0],
                  [0, 1, 1, 0],
                  [0, -1, 1, 0],
                  [0, 1, 0, -1]], np.float32)
G_np = np.array([[1, 0, 0],
                 [0.5, 0.5, 0.5],
                 [0.5, -0.5, 0.5],
                 [0, 0, 1]], np.float32)

# chain -> list of (slot, px): slot indexes the 5 host weight slots
# slots: 0:+W(px=0) 1:+W(px=1) 2:+W(px=2) 3:-W(px=2) 4:-W(px=3)
CHAINS = [[(0, 0), (1, 1), (2, 2)],   # ox=0
          [(1, 1), (3, 2), (4, 3)]]   # ox=1


def build_nc():
    nc = bacc.Bacc()
    xt_d = nc.declare_dram_parameter("xt", [4, P, 4, 2, 2, N], F8, isOutput=False)
    # weight slots: [g, slot, ki, py, kj, ko, co']
    wqk_d = nc.declare_dram_parameter("wqk", [2, 5, P, 4, 2, 2, 512], F8,
                                      isOutput=False)
    wv_d = nc.declare_dram_parameter("wv", [5, P, 4, 2, 2, 512], F8,
                                     isOutput=False)
    wproj_d = nc.declare_dram_parameter("wproj", [2, P, 2, C], F8, isOutput=False)
    bq8_d = nc.declare_dram_parameter("bq8", [P, 2, 2, 1], F8, isOutput=False)
    out_d = nc.declare_dram_parameter("out", [S, P, 4, NPIX], BF16, isOutput=True)

    with tile.TileContext(nc) as tc:
        with (
            tc.tile_pool(name="const", bufs=1) as constp,
            tc.tile_pool(name="qkv", bufs=1) as qkv,
        ):
            ones8 = constp.tile([P, 2, 16], F8, name="ones8")
            nc.vector.memset(ones8, 1.0)
            ones_row_f = constp.tile([1, P], F32, name="ones_row_f")
            nc.vector.memset(ones_row_f, 1.0 / float(1 << 21))

            # ---- weight/input pools (released after the conv) ----
            wpool = tc.alloc_tile_pool(name="wino", bufs=1)
            xt_sb, wv_sb, wqk_sb = {}, {}, {}
            for px in range(4):
                xt_sb[px] = wpool.tile([P, 4, 2, 2, N], F8, tag="xt", bufs=4,
                                       name=f"xt_{px}")
            for sl in range(5):
                wv_sb[sl] = wpool.tile([P, 4, 2, 2, 512], F8, tag="wv", bufs=5,
                                       name=f"wv_{sl}")
            for g in range(2):
                for sl in range(5):
                    wqk_sb[(g, sl)] = wpool.tile([P, 4, 2, 2, 512], F8,
                                                 tag="wqk", bufs=10,
                                                 name=f"wqk_{g}_{sl}")

            # All loads on the sync HWDGE queue as whole-tensor 1MB DMAs
            # (256KB-split transfers measured ~2x worse queue throughput, and
            # scalar-queue transfers crawled next to an active sync queue).
            # Interleave xt/wv in first-use order: the first conv unit's first
            # matmul needs only xt0+wv0.
            # first pair split in py-halves: the first conv unit's first
            # matmuls need only the py01 halves (~1MB instead of 2MB)
            nc.sync.dma_start(xt_sb[0][:, 0:2], xt_d[0, :, 0:2])
            nc.sync.dma_start(wv_sb[0][:, 0:2], wv_d[0, :, 0:2])
            nc.sync.dma_start(xt_sb[0][:, 2:4], xt_d[0, :, 2:4])
            nc.sync.dma_start(wv_sb[0][:, 2:4], wv_d[0, :, 2:4])
            for px in range(1, 4):
                nc.sync.dma_start(xt_sb[px], xt_d[px])
                nc.sync.dma_start(wv_sb[px], wv_d[px])
            nc.sync.dma_start(wv_sb[4], wv_d[4])
            for g in range(2):
                for sl in range(5):
                    nc.sync.dma_start(wqk_sb[(g, sl)], wqk_d[g, sl])
            wproj_sb = []
            for cj in range(2):
                t = constp.tile([P, 2, C], F8, tag="wproj", bufs=2,
                                name=f"wproj_{cj}")
                nc.sync.dma_start(t, wproj_d[cj])
                wproj_sb.append(t)
            bq8_sb = constp.tile([P, 2, 2, 1], F8, name="bq8_sb")
            nc.sync.dma_start(bq8_sb, bq8_d[:])

            # ---- persistent qkv outputs ----
            q8 = [qkv.tile([P, 2, 4 * N], F8, tag="q8", bufs=2, name=f"q8_{j}")
                  for j in range(2)]
            k8 = [qkv.tile([P, 2, 4 * N], F8, tag="k8", bufs=2, name=f"k8_{j}")
                  for j in range(2)]
            vT8 = {}
            for s in range(S):
                for oyox in range(4):
                    vT8[(s, oyox)] = qkv.tile([P, 2, 512], F8, tag="vt",
                                              bufs=S * 4, name=f"vt_{s}_{oyox}")
            cbias = [constp.tile([P, 8], F32, tag="cb", bufs=2, name=f"cb_{s}")
                     for s in range(S)]

            workp = tc.alloc_tile_pool(name="work", bufs=1)
            psc = tc.alloc_tile_pool(name="psc", bufs=8, space="PSUM")

            def conv_unit(make_lhsT, make_rhs, writes, split_s, tname):
                """Per (ocx|tc, ox) unit: 4 chain-GEMM banks M'[py] (each the
                px-chain accumulation, 6 MMs), then the py-contraction:
                  o[oy=0] = M'0+M'1+M'2 ; o[oy=1] = M'1-M'2-M'3
                writes[oy] = fp8 dst AP."""
                m = [psc.tile([P, N], F32, tag="m", name=f"m_{tname}_{py}")
                     for py in range(4)]
                for py in range(4):
                    for ci, (sl, px) in enumerate(make_lhsT["chain"]):
                        for kj in range(2):
                            nc.tensor.matmul(
                                m[py],
                                lhsT=make_lhsT["fn"](sl, px, py, kj),
                                rhs=make_rhs(sl, px, py, kj),
                                start=(ci == 0 and kj == 0),
                                stop=(ci == 2 and kj == 1),
                                perf_mode=DR)
                def v3(ap):
                    return (ap.rearrange("p (s r) -> p s r", s=2)
                            if split_s else ap)
                e1 = workp.tile([P, N], BF16, tag="e", bufs=6, name=f"e1_{tname}")
                e2 = workp.tile([P, N], BF16, tag="e", bufs=6, name=f"e2_{tname}")
                nc.scalar.copy(e1, m[1])
                nc.scalar.copy(e2, m[2])
                a = workp.tile([P, N], BF16, tag="u", bufs=6, name=f"a_{tname}")
                bb = workp.tile([P, N], BF16, tag="u", bufs=6, name=f"b_{tname}")
                nc.vector.tensor_add(a, m[0], e1)
                nc.vector.tensor_add(writes[0], v3(a), v3(e2))
                nc.gpsimd.tensor_sub(bb, e1, e2)
                nc.vector.tensor_sub(writes[1], v3(bb), m[3] if not split_s
                                     else m[3].rearrange("p (s r) -> p s r", s=2))
                return

            # ---- phase A: v conv (transposed GEMMs; signs fold into rhs) ----
            for tcx in range(4):   # tcx = s*2 + half
                s, half = divmod(tcx, 2)
                for ox, chain in enumerate(CHAINS):
                    conv_unit(
                        {"chain": chain,
                         "fn": lambda sl, px, py, kj, tcx=tcx: xt_sb[px][
                             :, py, kj, :, tcx * P:(tcx + 1) * P]},
                        lambda sl, px, py, kj: wv_sb[sl][:, py, kj],
                        [vT8[(s, oy * 2 + ox)][:, half] for oy in range(2)],
                        False, f"v_{tcx}_{ox}")

            # ---- phase B: q,k conv ----
            # Alternate folded (ocl 0,2) and pure-Winograd (ocl 1,3) columns:
            # pure units save 16 MMs per ocl (the px-savings the fold gave up)
            # and their vector-side drain hides under the adjacent folded
            # units' longer PE cover. Pure px=3 reuses weight slot 4 (-W3),
            # so its t-planes come out negated and the stage-2 ops that
            # consume them flip from sub to add.
            def v3q(ap):
                return ap.rearrange("p (s r) -> p s r", s=2)

            for g in range(2):
                dst = q8 if g == 0 else k8
                for ocl in range(4):
                    j, i = divmod(ocl, 2)
                    dr = dst[j].rearrange("p i (s o) -> p i s o", s=2)

                    def qdst(oyox):
                        return dr[:, i, :, oyox * T:(oyox + 1) * T]

                    if ocl % 2 == 0:   # folded column
                        for ox, chain in enumerate(CHAINS):
                            conv_unit(
                                {"chain": chain,
                                 "fn": lambda sl, px, py, kj, g=g, ocl=ocl:
                                     wqk_sb[(g, sl)][:, py, kj,
                                                     :, ocl * P:(ocl + 1) * P]},
                                lambda sl, px, py, kj: xt_sb[px][:, py, kj],
                                [qdst(oy * 2 + ox) for oy in range(2)],
                                True, f"qk_{g}_{ocl}_{ox}")
                        continue

                    # pure-Winograd column
                    tt = {}
                    for px in range(4):
                        sl = (0, 1, 2, 4)[px]
                        tn = f"qkp_{g}_{ocl}_{px}"
                        m = [psc.tile([P, N], F32, tag="m",
                                      name=f"m_{tn}_{py}") for py in range(4)]
                        for py in range(4):
                            for kj in range(2):
                                nc.tensor.matmul(
                                    m[py],
                                    lhsT=wqk_sb[(g, sl)][:, py, kj,
                                                         :, ocl * P:(ocl + 1) * P],
                                    rhs=xt_sb[px][:, py, kj],
                                    start=(kj == 0), stop=(kj == 1),
                                    perf_mode=DR)
                        e1 = workp.tile([P, N], BF16, tag="e", bufs=6,
                                        name=f"e1_{tn}")
                        e2 = workp.tile([P, N], BF16, tag="e", bufs=6,
                                        name=f"e2_{tn}")
                        nc.scalar.copy(e1, m[1])
                        nc.scalar.copy(e2, m[2])
                        a = workp.tile([P, N], BF16, tag="u", bufs=6,
                                       name=f"a_{tn}")
                        bb = workp.tile([P, N], BF16, tag="u", bufs=6,
                                        name=f"b_{tn}")
                        t0 = workp.tile([P, N], BF16, tag="t", bufs=16,
                                        name=f"t0_{tn}")
                        t1 = workp.tile([P, N], BF16, tag="t", bufs=16,
                                        name=f"t1_{tn}")
                        nc.vector.tensor_add(a, m[0], e1)
                        nc.vector.tensor_add(t0, a, e2)
                        nc.gpsimd.tensor_sub(bb, e1, e2)
                        nc.vector.tensor_sub(t1, bb, m[3])
                        tt[(0, px)] = t0
                        tt[(1, px)] = t1
                    for oy in range(2):
                        tn = f"qkp2_{g}_{ocl}_{oy}"
                        u0 = workp.tile([P, N], BF16, tag="u", bufs=6,
                                        name=f"u0_{tn}")
                        u1 = workp.tile([P, N], BF16, tag="u", bufs=6,
                                        name=f"u1_{tn}")
                        nc.gpsimd.tensor_add(u0, tt[(oy, 0)], tt[(oy, 1)])
                        nc.vector.tensor_add(qdst(oy * 2), v3q(u0),
                                             v3q(tt[(oy, 2)]))
                        nc.gpsimd.tensor_sub(u1, tt[(oy, 1)], tt[(oy, 2)])
                        # px3 t-plane is negated (slot 4 = -W3): add, not sub
                        nc.vector.tensor_add(qdst(oy * 2 + 1), v3q(u1),
                                             v3q(tt[(oy, 3)]))

            # ---- q-bias pre-exp term: c[m] = SEXP * (bq . k) ----
            for s in range(S):
                for mc in range(8):
                    oyox, half = divmod(mc, 2)
                    off = s * NPIX + oyox * T + half * P
                    cb = psc.tile([P, N], F32, tag="m", name=f"cb_{s}_{mc}")
                    for j in range(2):
                        nc.tensor.matmul(cb[:, 0:1],
                                         lhsT=k8[j][:, :, off:off + P],
                                         rhs=bq8_sb[:, j],
                                         start=(j == 0), stop=(j == 1),
                                         perf_mode=DR)
                    nc.scalar.mul(cbias[s][:, mc:mc + 1], cb[:, 0:1], SEXP)

            workp.release()
            wpool.release()
            psc.release()

            # ---- attention ----
            with (
                tc.tile_pool(name="attn", bufs=1) as attn,
                tc.tile_pool(name="stream", bufs=2) as stream,
                tc.tile_pool(name="psm", bufs=3, space="PSUM") as psm,
                tc.tile_pool(name="pss", bufs=1, space="PSUM") as pss,
            ):
                exps8 = {}
                for s in range(S):
                    for mj in range(4):
                        exps8[(s, mj)] = attn.tile([P, 2, NPIX], F8, tag="exps",
                                                   bufs=S * 4,
                                                   name=f"exps_{s}_{mj}")
                # scores for both samples (PE stays busy while exps drain)
                for s in range(S):
                    for mc in range(8):
                        oyox, half = divmod(mc, 2)
                        off = s * NPIX + oyox * T + half * P
                        ps = psm.tile([P, NPIX], F32, tag="mm2",
                                      name=f"ps_sc_{s}_{mc}")
                        for j in range(2):
                            for nt in range(2):
                                nc.tensor.matmul(
                                    ps[:, nt * 512:(nt + 1) * 512],
                                    lhsT=k8[j][:, :, off:off + P],
                                    rhs=q8[j][:, :, s * NPIX + nt * 512:
                                              s * NPIX + (nt + 1) * 512],
                                    start=(j == 0), stop=(j == 1),
                                    perf_mode=DR)
                        nc.scalar.activation(
                            exps8[(s, mc // 2)][:, mc % 2, :], ps, EXP,
                            scale=SEXP, bias=cbias[s][:, mc:mc + 1])

                for s in range(S):
                    # row sums + reciprocal
                    r_sb = stream.tile([1, NPIX], F32, tag="r", bufs=2,
                                       name=f"r_{s}")
                    ps_sum = pss.tile([1, NPIX], F32, tag="sum",
                                      name=f"ps_sum_{s}")
                    for mj in range(4):
                        for nt in range(2):
                            nc.tensor.matmul(
                                ps_sum[:, nt * 512:(nt + 1) * 512],
                                lhsT=ones8[:, :, 0:1],
                                rhs=exps8[(s, mj)][:, :, nt * 512:(nt + 1) * 512],
                                start=(mj == 0), stop=(mj == 3),
                                perf_mode=DR)
                    nc.vector.reciprocal_approx_fast(out=r_sb, in_=ps_sum)

                    # h_unT, staged fp8 at 1/32
                    hN = [attn.tile([P, 2, NPIX], F8, tag="hn", bufs=2,
                                    name=f"hn_{s}_{cj}") for cj in range(2)]
                    for cc in range(4):
                        ps_h = psm.tile([P, NPIX], F32, tag="mm2",
                                        name=f"ps_h_{s}_{cc}")
                        for mj in range(4):
                            for nt in range(2):
                                nc.tensor.matmul(
                                    ps_h[:, nt * 512:(nt + 1) * 512],
                                    lhsT=vT8[(s, mj)][:, :, cc * P:(cc + 1) * P],
                                    rhs=exps8[(s, mj)][:, :,
                                                       nt * 512:(nt + 1) * 512],
                                    start=(mj == 0), stop=(mj == 3),
                                    perf_mode=DR)
                        nc.scalar.mul(hN[cc // 2][:, cc % 2, :], ps_h, 1.0 / 32.0)

                    # broadcast 1/sums across partitions (K=1 matmul)
                    ps_b = psm.tile([P, NPIX], F32, tag="mm2", name=f"ps_rb_{s}")
                    for nt in range(2):
                        nc.tensor.matmul(ps_b[:, nt * 512:(nt + 1) * 512],
                                         lhsT=ones_row_f,
                                         rhs=r_sb[:, nt * 512:(nt + 1) * 512],
                                         start=True, stop=True)
                    rbc = stream.tile([P, NPIX], F32, tag="rbc", bufs=2,
                                      name=f"rbc_{s}")
                    nc.scalar.copy(out=rbc, in_=ps_b)

                    # proj + normalize + store
                    o_t = stream.tile([P, 4, NPIX], BF16, tag="ostage", bufs=2,
                                      name=f"o_{s}")
                    for oc in range(4):
                        ps_p = psm.tile([P, NPIX], F32, tag="mm2",
                                        name=f"ps_p_{s}_{oc}")
                        for cj in range(2):
                            for nt in range(2):
                                nc.tensor.matmul(
                                    ps_p[:, nt * 512:(nt + 1) * 512],
                                    lhsT=wproj_sb[cj][:, :, oc * P:(oc + 1) * P],
                                    rhs=hN[cj][:, :, nt * 512:(nt + 1) * 512],
                                    start=(cj == 0), stop=(cj == 1),
                                    perf_mode=DR)
                        nc.vector.tensor_mul(out=o_t[:, oc], in0=ps_p, in1=rbc)
                        nc.scalar.dma_start(out_d[s, :, oc], o_t[:, oc])

    nc.finalize()
    return nc


def prep_inputs(x, w_qkv, b_qkv, w_proj):
    """Host-side Winograd transforms + fp8 packing. Returns full-batch arrays."""
    e4 = ml_dtypes.float8_e4m3
    wt = np.einsum('pa,oiab,qb->pqio', G_np, w_qkv, G_np) * 32.0
    wt8 = wt.astype(e4).astype(np.float32)   # [4py, 4px, 512ci, 1536co]
    # 5 slots along px: +px0 +px1 +px2 -px2 -px3 (fp8 negation is exact)
    slots = np.stack([wt8[:, 0], wt8[:, 1], wt8[:, 2],
                      -wt8[:, 2], -wt8[:, 3]], axis=0)  # [5, 4py, ci, co]
    # -> [g, slot, ki, py, kj, ko, co'] / [slot, ki, py, kj, ko, co']
    sq = slots[:, :, :, :1024].reshape(5, 4, 2, 2, P, 2, 512)
    wqk = np.ascontiguousarray(sq.transpose(5, 0, 4, 1, 2, 3, 6)).astype(e4)
    sv = slots[:, :, :, 1024:].reshape(5, 4, 2, 2, P, 512)
    wv = np.ascontiguousarray(sv.transpose(0, 4, 1, 2, 3, 5)).astype(e4)

    xpad = np.zeros((B, C, H + 2, W + 2), np.float32)
    xpad[:, :, 1:H + 1, 1:W + 1] = x
    s0, s1 = xpad.strides[-2:]
    win = np.lib.stride_tricks.as_strided(
        xpad, (B, C, 16, 16, 4, 4),
        xpad.strides[:2] + (2 * s0, 2 * s1, s0, s1))
    xt = np.einsum('pa,qb,ncijab->pqncij', Bt_np, Bt_np, win)
    xt8 = xt.astype(e4)   # [4py, 4px, B, C, 16, 16]

    bq8 = np.ascontiguousarray(
        (b_qkv[:512] * 32.0).reshape(2, 2, P).transpose(2, 0, 1)[..., None]
    ).astype(e4)
    wproj8 = np.ascontiguousarray(
        (w_proj[:, :, 0, 0].T * float(1 << 21))
        .reshape(2, 2, P, C).transpose(0, 2, 1, 3)).astype(e4)
    return wqk, wv, xt8, bq8, wproj8


def core_inputs(xt8, core):
    """Per-core X-tilde: [px, ki, py, kj, ko, (s,t)] fp8."""
    sl = xt8[:, :, core * S:(core + 1) * S]          # [4,4,S,C,16,16]
    arr = sl.reshape(4, 4, S, 2, 2, P, T)            # py,px,s,kj,ko,ki,t
    return np.ascontiguousarray(
        arr.transpose(1, 5, 0, 3, 4, 2, 6).reshape(4, P, 4, 2, 2, N))


# device pixel index n = oyox*256 + ty*16 + tx  ->  image pixel
_n = np.arange(NPIX)
_oyox, _t = _n >> 8, _n & 255
_PIX = (2 * (_t >> 4) + (_oyox >> 1)) * 32 + 2 * (_t & 15) + (_oyox & 1)


def kernel(x, w_qkv, b_qkv, w_proj, b_proj, gn_gamma=None, gn_beta=None):
    global LAST_EXEC_NS, _CACHED
    x = np.asarray(x, np.float32)
    w_qkv = np.asarray(w_qkv, np.float32)
    b_qkv = np.asarray(b_qkv, np.float32)
    w_proj = np.asarray(w_proj, np.float32)
    b_proj = np.asarray(b_proj, np.float32)

    if _CACHED is None:
        _CACHED = build_nc()
    nc = _CACHED

    wqk, wv, xt8, bq8, wproj8 = prep_inputs(x, w_qkv, b_qkv, w_proj)
    in_maps = []
    for core in range(NCORES):
        in_maps.append({
            "xt": core_inputs(xt8, core),
            "wqk": wqk,
            "wv": wv,
            "wproj": wproj8,
            "bq8": bq8,
        })

    res = run_bass_kernel_spmd(nc, in_maps, list(range(NCORES)), trace=TRACE)
    LAST_EXEC_NS = res.exec_time_ns
    h = np.stack([np.asarray(res.results[c]["out"], np.float32)
                  for c in range(NCORES)])            # [8, S, P, 4, NPIX]
    h = h.reshape(B, P, 4, NPIX).transpose(0, 2, 1, 3).reshape(B, C, NPIX)
    himg = np.empty_like(h)
    himg[:, :, _PIX] = h
    himg = himg.reshape(B, C, H, W)

    const = b_proj + w_proj[:, :, 0, 0] @ b_qkv[1024:]
    out = x + himg + const[None, :, None, None]
    return np.ascontiguousarray(out).astype(np.float32, copy=False)
